# revision 1
# baseline (speedup 1.0000x reference)
"""Trainium2 Bass kernel for causal multi-head attention with RoPE.

Problem: B=2, T=2048, C=2048, H=16, D=128.
Sharding over 8 NeuronCores: batch (2) x head-group (4 heads each); the host
sums the 4 per-head-group partials per batch and adds bo' = bo + bv @ Wo.T
(the v-bias commutes through softmax since rows sum to 1).

v2 design notes:
- All matmuls in float32r (fp32 storage, reduced-precision single-pass PE
  matmul, ~4x faster than fp32, measured rel err ~1.5e-4 at K=2048).
- Transposed-everything layout: xT [C,T], qT/kT [D,T], v [T,D], attnT [D,T],
  out [T,C]; every matmul contracts over partitions, no activation transposes.
- Max-free softmax (scores bounded ~6): scores computed PRE-TRANSPOSED as
  ST = K^T-block x Q-group ([T_k=128, T_q=512] tiles), exp'd directly; row
  sums via a ones-vector matmul accumulated in PSUM; normalization applied to
  attnT via partition-broadcast reciprocal multiply. No PE transposes at all.
- Causality at 128-block granularity: blocks above the diagonal skipped,
  diagonal 128x128 sub-block masked additively, partial-width matmuls
  elsewhere on the diagonal.
- RoPE rotate-half via a PE matmul with a +-1 permutation matrix; 1/sqrt(D)
  folded into Wq/bq on the host.
"""

import math
import sys

import numpy as np

for _p in ("/opt/trn_rl_repo", "/root/.axon_site/_ro/trn_rl_repo"):
    if _p not in sys.path:
        sys.path.append(_p)

import concourse.bacc as bacc
import concourse.bass as bass
import concourse.mybir as mybir
import concourse.tile as tile
from contextlib import ExitStack

F32 = mybir.dt.float32
F32R = mybir.dt.float32r
AF = mybir.ActivationFunctionType
ALU = mybir.AluOpType
AX = mybir.AxisListType

B, T, C = 2, 2048, 2048
H, D = 16, 128
THETA = 10000.0
NEG = -1e9

N_CORES = 8
GROUPS = 4          # head groups (other shard axis is batch)
HPC = H // GROUPS   # heads per core
GW = 512            # T-group width (q-group / proj chunk)


def build_core_nc(T_=T, C_=C, hpc=HPC, debug=False):
    KT = C_ // 128          # contraction k-tiles
    QT = T_ // 128          # 128-wide T tiles
    G = T_ // GW            # 512-wide T groups
    PASSES = hpc // 2

    nc = bacc.Bacc(None, target_bir_lowering=False, debug=debug)

    xT = nc.dram_tensor("xT", [C_, T_], F32R, kind="ExternalInput")
    wqT = nc.dram_tensor("wqT", [C_, hpc * 128], F32R, kind="ExternalInput")
    wkT = nc.dram_tensor("wkT", [C_, hpc * 128], F32R, kind="ExternalInput")
    wvT = nc.dram_tensor("wvT", [C_, hpc * 128], F32R, kind="ExternalInput")
    woT = nc.dram_tensor("woT", [hpc * 128, C_], F32R, kind="ExternalInput")
    bq = nc.dram_tensor("bq", [hpc * 128], F32, kind="ExternalInput")
    bk = nc.dram_tensor("bk", [hpc * 128], F32, kind="ExternalInput")
    cosT = nc.dram_tensor("cosT", [128, T_], F32, kind="ExternalInput")
    sinT = nc.dram_tensor("sinT", [128, T_], F32, kind="ExternalInput")
    maskT = nc.dram_tensor("maskT", [128, 128], F32, kind="ExternalInput")
    rt = nc.dram_tensor("rt", [128, 128], F32R, kind="ExternalInput")
    ones = nc.dram_tensor("ones", [128, 1], F32R, kind="ExternalInput")
    out = nc.dram_tensor("out", [T_, C_], F32, kind="ExternalOutput")

    with tile.TileContext(nc) as tc, ExitStack() as top:
        const = top.enter_context(tc.tile_pool(name="const", bufs=1))
        bq_sb = const.tile([128, hpc], F32, name="bq_sb")
        nc.sync.dma_start(bq_sb[:], bq.rearrange("(h d) -> d h", d=128))
        bk_sb = const.tile([128, hpc], F32, name="bk_sb")
        nc.sync.dma_start(bk_sb[:], bk.rearrange("(h d) -> d h", d=128))
        maskT_sb = const.tile([128, 128], F32, name="maskT_sb")
        nc.sync.dma_start(maskT_sb[:], maskT[:, :])
        rt_sb = const.tile([128, 128], F32R, name="rt_sb")
        nc.sync.dma_start(rt_sb[:], rt[:, :])
        ones_sb = const.tile([128, 1], F32R, name="ones_sb")
        nc.sync.dma_start(ones_sb[:], ones[:, :])

        attnp = top.enter_context(tc.tile_pool(name="attnp", bufs=1))
        attnT = attnp.tile([128, hpc, T_], F32R, name="attnT")

        with ExitStack() as ph:
            xp = ph.enter_context(tc.tile_pool(name="xp", bufs=12))
            wp = ph.enter_context(tc.tile_pool(name="wp", bufs=1))
            kv = ph.enter_context(tc.tile_pool(name="kv", bufs=1))
            qp = ph.enter_context(tc.tile_pool(name="qp", bufs=2))
            raw = ph.enter_context(tc.tile_pool(name="raw", bufs=2))
            ptp = ph.enter_context(tc.tile_pool(name="ptp", bufs=3))
            csp = ph.enter_context(tc.tile_pool(name="csp", bufs=1))
            nrm = ph.enter_context(tc.tile_pool(name="nrm", bufs=2))
            smp = ph.enter_context(tc.tile_pool(name="smp", bufs=1))
            acc = ph.enter_context(tc.tile_pool(name="acc", bufs=2, space="PSUM"))
            stp = ph.enter_context(tc.tile_pool(name="stp", bufs=3, space="PSUM"))
            avp = ph.enter_context(tc.tile_pool(name="avp", bufs=1, space="PSUM"))
            onp = ph.enter_context(tc.tile_pool(name="onp", bufs=1, space="PSUM"))

            for p in range(PASSES):
                pcols = slice(p * 256, (p + 1) * 256)
                wq_sb = wp.tile([128, KT, 256], F32R, tag="wq", name=f"wq_{p}")
                nc.sync.dma_start(
                    wq_sb[:], wqT[:, pcols].rearrange("(ko ki) n -> ki ko n", ki=128)
                )
                wk_sb = wp.tile([128, KT, 256], F32R, tag="wk", name=f"wk_{p}")
                nc.sync.dma_start(
                    wk_sb[:], wkT[:, pcols].rearrange("(ko ki) n -> ki ko n", ki=128)
                )
                wv_sb = wp.tile([128, KT, 256], F32R, tag="wv", name=f"wv_{p}")
                nc.sync.dma_start(
                    wv_sb[:], wvT[:, pcols].rearrange("(ko ki) n -> ki ko n", ki=128)
                )
                kT_sb = kv.tile([128, 2, T_], F32R, tag="kT", name=f"kT_{p}")
                v_sb = kv.tile([128, QT, 256], F32R, tag="v", name=f"v_{p}")
                qts = {}

                def proj_chunk(g, p=p, pcols=pcols, wq_sb=wq_sb, wk_sb=wk_sb,
                               wv_sb=wv_sb, kT_sb=kT_sb, v_sb=v_sb, qts=qts):
                    gcols = slice(g * GW, (g + 1) * GW)
                    x_subs = []
                    for kq in range(KT // 2):
                        xs = xp.tile([128, 2, GW], F32R, tag="x",
                                     name=f"x_{p}_{g}_{kq}")
                        nc.sync.dma_start(
                            xs[:],
                            xT[kq * 256 : (kq + 1) * 256, gcols].rearrange(
                                "(ko ki) t -> ki ko t", ki=128
                            ),
                        )
                        x_subs.append(xs)
                    cos_sb = csp.tile([128, GW], F32, tag="cos", name=f"cos_{p}_{g}")
                    nc.sync.dma_start(cos_sb[:], cosT[:, gcols])
                    sin_sb = csp.tile([128, GW], F32, tag="sin", name=f"sin_{p}_{g}")
                    nc.sync.dma_start(sin_sb[:], sinT[:, gcols])

                    qT_sb = qp.tile([128, 2, GW], F32R, tag="qT", name=f"qT_{p}_{g}")
                    qts[g] = qT_sb

                    raws = {}
                    for wsb, bias_sb, is_q in (
                        (wq_sb, bq_sb, True),
                        (wk_sb, bk_sb, False),
                    ):
                        psums = [
                            acc.tile([128, GW], F32, tag="acc",
                                     name=f"pp_{p}_{g}_{is_q}_{hl}")
                            for hl in range(2)
                        ]
                        for kk in range(KT):
                            for hl in range(2):
                                nc.tensor.matmul(
                                    psums[hl][:],
                                    wsb[:, kk, hl * 128 : (hl + 1) * 128],
                                    x_subs[kk // 2][:, kk % 2, :],
                                    start=(kk == 0),
                                    stop=(kk == KT - 1),
                                )
                        for hl in range(2):
                            h = p * 2 + hl
                            q_raw = raw.tile([128, GW], F32R, tag="raw",
                                             name=f"raw_{p}_{g}_{is_q}_{hl}")
                            nc.scalar.activation(
                                q_raw[:], psums[hl][:], AF.Identity,
                                bias=bias_sb[:, h : h + 1],
                            )
                            raws[(is_q, hl)] = q_raw

                    def rope_pair(is_q, p=p, g=g, gcols=gcols, raws=raws,
                                  cos_sb=cos_sb, sin_sb=sin_sb,
                                  qT_sb=qT_sb, kT_sb=kT_sb):
                        for hl in range(2):
                            q_raw = raws[(is_q, hl)]
                            rps = stp.tile([128, GW], F32, tag="st",
                                           name=f"rot_{p}_{g}_{is_q}_{hl}")
                            nc.tensor.matmul(rps[:], rt_sb[:], q_raw[:],
                                             start=True, stop=True)
                            tcos = raw.tile([128, GW], F32, tag="tcos")
                            nc.vector.tensor_tensor(
                                tcos[:], q_raw[:], cos_sb[:], ALU.mult
                            )
                            usin = raw.tile([128, GW], F32, tag="usin")
                            nc.vector.tensor_tensor(
                                usin[:], rps[:], sin_sb[:], ALU.mult
                            )
                            dest = (
                                qT_sb[:, hl, :] if is_q else kT_sb[:, hl, gcols]
                            )
                            nc.gpsimd.tensor_tensor(
                                dest, tcos[:], usin[:], ALU.add
                            )

                    rope_pair(True)

                    # v projection: two T-tiles at a time, N=256
                    for tpair in range(2):
                        vps = [
                            acc.tile([128, 256], F32, tag="acc",
                                     name=f"vp_{p}_{g}_{tpair}_{ti}")
                            for ti in range(2)
                        ]
                        for kk in range(KT):
                            for ti in range(2):
                                tloc = tpair * 2 + ti
                                nc.tensor.matmul(
                                    vps[ti][:],
                                    x_subs[kk // 2][
                                        :, kk % 2, tloc * 128 : (tloc + 1) * 128
                                    ],
                                    wv_sb[:, kk, :],
                                    start=(kk == 0),
                                    stop=(kk == KT - 1),
                                )
                        for ti in range(2):
                            tt = g * 4 + tpair * 2 + ti
                            nc.scalar.copy(v_sb[:, tt, :], vps[ti][:])
                        if tpair == 0:
                            rope_pair(False)

                def attn_group(g, p=p, kT_sb=kT_sb, v_sb=v_sb, qts=qts):
                    qT_sb = qts[g]
                    for hl in range(2):
                        h = p * 2 + hl
                        av = avp.tile([128, GW], F32, tag="av",
                                      name=f"av_{p}_{g}_{hl}")
                        ons = onp.tile([1, GW], F32, tag="on",
                                       name=f"on_{p}_{g}_{hl}")
                        nblocks = 4 * g + 4

                        def emit_st(j, p=p, g=g, hl=hl, kT_sb=kT_sb,
                                    qT_sb=qT_sb):
                            di = j - 4 * g
                            c0 = di * 128 if di >= 0 else 0
                            st = stp.tile([128, GW], F32, tag="st",
                                          name=f"st_{p}_{g}_{hl}_{j}")
                            nc.tensor.matmul(
                                st[:, c0:GW],
                                kT_sb[:, hl, j * 128 : (j + 1) * 128],
                                qT_sb[:, hl, c0:GW],
                                start=True,
                                stop=True,
                            )
                            if di >= 0:
                                nc.vector.tensor_tensor(
                                    st[:, c0 : c0 + 128],
                                    st[:, c0 : c0 + 128],
                                    maskT_sb[:],
                                    ALU.add,
                                )
                            pt = ptp.tile([128, GW], F32R, tag="pt")
                            nc.scalar.activation(
                                pt[:, c0:GW], st[:, c0:GW], AF.Exp
                            )
                            return c0, pt

                        def emit_consume(j, c0, pt, nblocks=nblocks,
                                         hl=hl, av=av, ons=ons, v_sb=v_sb):
                            nc.tensor.matmul(
                                ons[0:1, c0:GW],
                                ones_sb[:],
                                pt[:, c0:GW],
                                start=(j == 0),
                                stop=(j == nblocks - 1),
                            )
                            nc.tensor.matmul(
                                av[:, c0:GW],
                                v_sb[:, j, hl * 128 : (hl + 1) * 128],
                                pt[:, c0:GW],
                                start=(j == 0),
                                stop=(j == nblocks - 1),
                            )

                        pending = []
                        for j in range(nblocks):
                            pending.append((j, *emit_st(j)))
                            if len(pending) > 2:
                                emit_consume(*pending.pop(0))
                        for item in pending:
                            emit_consume(*item)
                        gcols = slice(g * GW, (g + 1) * GW)
                        nc.scalar.copy(attnT[:, h, gcols], av[:])
                        on_sb = smp.tile([1, GW], F32, tag="on_sb")
                        nc.scalar.copy(on_sb[0:1, :], ons[0:1, :])
                        scr = smp.tile([1, GW], F32, tag="scr")
                        ri1 = smp.tile([1, GW], F32, tag="ri1")
                        nc.vector.reciprocal_approx_accurate(
                            ri1[0:1, :], on_sb[0:1, :], scr[0:1, :]
                        )
                        ri = nrm.tile([128, GW], F32, tag="ri")
                        nc.gpsimd.partition_broadcast(ri[:], ri1[0:1, :])
                        nc.vector.tensor_tensor(
                            attnT[:, h, gcols], attnT[:, h, gcols], ri[:],
                            ALU.mult,
                        )

                proj_chunk(0)
                for g in range(G):
                    if g + 1 < G:
                        proj_chunk(g + 1)
                    attn_group(g)

        # output projection: out[t, c] = sum_dloc attnT[dloc, t] * woT[dloc, c]
        with ExitStack() as oph:
            wop = oph.enter_context(tc.tile_pool(name="wop", bufs=1))
            ops = oph.enter_context(tc.tile_pool(name="ops", bufs=4, space="PSUM"))
            outp = oph.enter_context(tc.tile_pool(name="outp", bufs=3))
            wo_sb = wop.tile([128, hpc, C_], F32R, name="wo_sb")
            nc.sync.dma_start(
                wo_sb[:], woT.rearrange("(ho hi) c -> hi ho c", hi=128)
            )
            for tt in range(QT):
                for ncol in range(C_ // 512):
                    op = ops.tile([128, 512], F32, tag="op")
                    for kc in range(hpc):
                        nc.tensor.matmul(
                            op[:],
                            attnT[:, kc, tt * 128 : (tt + 1) * 128],
                            wo_sb[:, kc, ncol * 512 : (ncol + 1) * 512],
                            start=(kc == 0),
                            stop=(kc == hpc - 1),
                        )
                    osb = outp.tile([128, 512], F32, tag="osb")
                    nc.scalar.copy(osb[:], op[:])
                    nc.sync.dma_start(
                        out[tt * 128 : (tt + 1) * 128, ncol * 512 : (ncol + 1) * 512],
                        osb[:],
                    )

    nc.compile()
    return nc


def _rope_tables(T_, theta=THETA):
    inv = 1.0 / (theta ** (np.arange(0, D, 2, dtype=np.float64) / D))
    t = np.arange(T_, dtype=np.float64)
    fr = np.outer(t, inv)
    emb = np.concatenate([fr, fr], axis=1)
    return (
        np.cos(emb).T.astype(np.float32).copy(),
        np.sin(emb).T.astype(np.float32).copy(),
    )


def _maskT():
    tk = np.arange(128)[:, None]
    c = np.arange(128)[None, :]
    return np.where(c >= tk, 0.0, NEG).astype(np.float32)


def _rot_T():
    R = np.zeros((128, 128), dtype=np.float32)
    half = D // 2
    R[np.arange(half), np.arange(half) + half] = -1.0
    R[np.arange(half) + half, np.arange(half)] = 1.0
    return R.T.copy()


def prep_inputs(x, Wq, bq, Wk, bk, Wv, bv, Wo, bo):
    scale = 1.0 / math.sqrt(D)
    cosT, sinT = _rope_tables(T)
    maskT = _maskT()
    rt = _rot_T()
    ones = np.ones((128, 1), dtype=np.float32)
    xT = [np.ascontiguousarray(x[b].T) for b in range(B)]
    in_maps = []
    for c in range(N_CORES):
        b, g = c // GROUPS, c % GROUPS
        rows = slice(g * HPC * D, (g + 1) * HPC * D)
        in_maps.append(
            {
                "xT": xT[b],
                "wqT": np.ascontiguousarray((Wq[rows] * scale).T),
                "wkT": np.ascontiguousarray(Wk[rows].T),
                "wvT": np.ascontiguousarray(Wv[rows].T),
                "woT": np.ascontiguousarray(Wo[:, rows].T),
                "bq": np.ascontiguousarray(bq[rows] * scale),
                "bk": np.ascontiguousarray(bk[rows]),
                "cosT": cosT,
                "sinT": sinT,
                "maskT": maskT,
                "rt": rt,
                "ones": ones,
            }
        )
    bo_eff = (bo + bv @ Wo.T).astype(np.float32)
    return in_maps, bo_eff


_NC_CACHE = {}


def get_nc():
    if "nc" not in _NC_CACHE:
        _NC_CACHE["nc"] = build_core_nc()
    return _NC_CACHE["nc"]


def kernel(x, Wq, bq, Wk, bk, Wv, bv, Wo, bo):
    x = np.asarray(x, dtype=np.float32)
    args = [np.asarray(a, dtype=np.float32) for a in (Wq, bq, Wk, bk, Wv, bv, Wo, bo)]
    in_maps, bo_eff = prep_inputs(x, *args)
    nc = get_nc()

    from concourse.bass_utils import run_bass_kernel_spmd

    res = run_bass_kernel_spmd(nc, in_maps, core_ids=list(range(N_CORES))).results

    out = np.empty((B, T, C), dtype=np.float32)
    for b in range(B):
        acc_ = res[b * GROUPS]["out"].astype(np.float32).copy()
        for g in range(1, GROUPS):
            acc_ += res[b * GROUPS + g]["out"]
        out[b] = acc_ + bo_eff
    return out



# revision 2
# speedup vs baseline: 1.0704x; 1.0704x over previous
"""Trainium2 Bass kernel for causal multi-head attention with RoPE.

Problem: B=2, T=2048, C=2048, H=16, D=128.
Sharding over 8 NeuronCores: batch (2) x head-group (4 heads each); the host
sums the 4 per-head-group partials per batch and adds bo' = bo + bv @ Wo.T
(the v-bias commutes through softmax since rows sum to 1).

v3 design notes (vs v2 which ran fp32r at ~73% PE occupancy / 526us):
- All matmuls bf16 (1 cyc/col at 2.4GHz warm; fp8 is numerically fatal here:
  the softmax is peaked enough that even fp8 x alone gives 4% output error).
  bf16 enables FWL fast weight loads so LDWEIGHTS hides under the matmuls.
- Phased structure to keep the PE stream dense (HAM clock-gate at 2.4GHz):
  A-K, A-Q projections (weights stationary per (head, k-slab), x moving,
  512-col chunks), A-V (x stationary per t-tile, wv moving), then attention
  with O-proj(g-1) interleaved between attention groups.
- x resident in SBUF as bf16 (8MB), per-k-slab DMAs so the first matmul only
  waits on slab 0.
- RoPE entirely off the PE: rotate-half via SBUF->SBUF DMA partition swap
  (sign folded into the sin table), then 2 DVE mults + 1 GPSIMD add.
- Max-free softmax, scores pre-transposed ST=[tk,tq] as v2; exp batched in
  [128,1024] pairs on the scalar engine (halves the 352-cycle/instr
  overhead); ones-vector matmul row-sums accumulated in PSUM.
- PSUM: phase B uses exactly 8 banks (ST pairs 2x2 + av 1 + ones 1 +
  oproj 2). av is evacuated to SBUF immediately so the next head's
  accumulation isn't blocked by the normalize chain.
"""

import math
import sys

import numpy as np

for _p in ("/opt/trn_rl_repo", "/root/.axon_site/_ro/trn_rl_repo"):
    if _p not in sys.path:
        sys.path.append(_p)

import ml_dtypes

import concourse.bacc as bacc
import concourse.bass as bass
import concourse.mybir as mybir
import concourse.tile as tile
from contextlib import ExitStack

F32 = mybir.dt.float32
BF = mybir.dt.bfloat16
AF = mybir.ActivationFunctionType
ALU = mybir.AluOpType

B, T, C = 2, 2048, 2048
H, D = 16, 128
THETA = 10000.0
NEG = -1e9

N_CORES = 8
GROUPS = 4          # head groups (other shard axis is batch)
HPC = H // GROUPS   # heads per core
KT = C // 128       # contraction k-slabs
GW = 512            # T-group width
NG = T // GW        # attention groups
NT = T // 128       # 128-wide t-tiles
BF_NP = ml_dtypes.bfloat16


def build_core_nc(debug=False):
    nc = bacc.Bacc(None, target_bir_lowering=False, debug=debug)

    xT = nc.dram_tensor("xT", [C, T], BF, kind="ExternalInput")
    wqT = nc.dram_tensor("wqT", [C, HPC * 128], BF, kind="ExternalInput")
    wkT = nc.dram_tensor("wkT", [C, HPC * 128], BF, kind="ExternalInput")
    wvT = nc.dram_tensor("wvT", [C, HPC * 128], BF, kind="ExternalInput")
    woT = nc.dram_tensor("woT", [HPC * 128, C], BF, kind="ExternalInput")
    bq = nc.dram_tensor("bq", [HPC * 128], F32, kind="ExternalInput")
    bk = nc.dram_tensor("bk", [HPC * 128], F32, kind="ExternalInput")
    cosT = nc.dram_tensor("cosT", [128, T], BF, kind="ExternalInput")
    sinM = nc.dram_tensor("sinM", [128, T], BF, kind="ExternalInput")
    maskT = nc.dram_tensor("maskT", [128, 128], F32, kind="ExternalInput")
    ones = nc.dram_tensor("ones", [128, 1], BF, kind="ExternalInput")
    out = nc.dram_tensor("out", [T, C], F32, kind="ExternalOutput")

    with tile.TileContext(nc) as tc, ExitStack() as top:
        const = top.enter_context(tc.tile_pool(name="const", bufs=1))
        big = top.enter_context(tc.tile_pool(name="big", bufs=1))

        bq_sb = const.tile([128, HPC], F32, name="bq_sb")
        nc.sync.dma_start(bq_sb[:], bq.rearrange("(h d) -> d h", d=128))
        bk_sb = const.tile([128, HPC], F32, name="bk_sb")
        nc.sync.dma_start(bk_sb[:], bk.rearrange("(h d) -> d h", d=128))
        mask_sb = const.tile([128, 128], F32, name="mask_sb")
        nc.sync.dma_start(mask_sb[:], maskT[:, :])
        ones_sb = const.tile([128, 1], BF, name="ones_sb")
        nc.sync.dma_start(ones_sb[:], ones[:, :])

        # resident tensors
        x_sb = big.tile([128, KT, T], BF, name="x_sb")
        for k in range(KT):
            nc.sync.dma_start(x_sb[:, k, :], xT[k * 128 : (k + 1) * 128, :])
        cos_sb = const.tile([128, T], BF, name="cos_sb")
        nc.sync.dma_start(cos_sb[:], cosT[:, :])
        sin_sb = const.tile([128, T], BF, name="sin_sb")
        nc.sync.dma_start(sin_sb[:], sinM[:, :])
        wv_sb = big.tile([128, KT, HPC * 128], BF, name="wv_sb")
        nc.sync.dma_start(
            wv_sb[:], wvT.rearrange("(ko ki) n -> ki ko n", ki=128)
        )
        wo_sb = big.tile([128, HPC, C], BF, name="wo_sb")
        nc.sync.dma_start(
            wo_sb[:], woT.rearrange("(ho hi) c -> hi ho c", hi=128)
        )
        qT = big.tile([128, HPC, T], BF, name="qT")
        kTt = big.tile([128, HPC, T], BF, name="kTt")
        v_sb = big.tile([128, NT, HPC * 128], BF, name="v_sb")

        # ---- Phase A: projections ----
        with ExitStack() as pa:
            wt = pa.enter_context(tc.tile_pool(name="wt", bufs=2))
            qkp = pa.enter_context(tc.tile_pool(name="qkp", bufs=3, space="PSUM"))
            raw = pa.enter_context(tc.tile_pool(name="raw", bufs=3))

            for wdram, bias_sb, dstT, nm in (
                (wkT, bk_sb, kTt, "k"),
                (wqT, bq_sb, qT, "q"),
            ):
                for h in range(HPC):
                    wtile = wt.tile([128, KT, 128], BF, tag="w", name=f"w_{nm}{h}")
                    nc.sync.dma_start(
                        wtile[:],
                        wdram[:, h * 128 : (h + 1) * 128].rearrange(
                            "(ko ki) n -> ki ko n", ki=128
                        ),
                    )
                    for ch in range(T // GW):
                        cols = slice(ch * GW, (ch + 1) * GW)
                        ps = qkp.tile([128, GW], F32, tag="qk")
                        for k in range(KT):
                            nc.tensor.matmul(
                                ps[:],
                                wtile[:, k, :],
                                x_sb[:, k, cols],
                                start=(k == 0),
                                stop=(k == KT - 1),
                            )
                        rawt = raw.tile([128, GW], BF, tag="raw")
                        nc.scalar.activation(
                            rawt[:], ps[:], AF.Identity, bias=bias_sb[:, h : h + 1]
                        )
                        swap = raw.tile([128, GW], BF, tag="swap")
                        nc.sync.dma_start(swap[0:64, :], rawt[64:128, :])
                        nc.sync.dma_start(swap[64:128, :], rawt[0:64, :])
                        t1 = raw.tile([128, GW], BF, tag="t1")
                        nc.vector.tensor_tensor(
                            t1[:], rawt[:], cos_sb[:, cols], ALU.mult
                        )
                        t2 = raw.tile([128, GW], BF, tag="t2")
                        nc.vector.tensor_tensor(
                            t2[:], swap[:], sin_sb[:, cols], ALU.mult
                        )
                        nc.gpsimd.tensor_tensor(
                            dstT[:, h, cols], t1[:], t2[:], ALU.add
                        )

            for tt in range(NT):
                ps = qkp.tile([128, HPC * 128], F32, tag="v")
                for k in range(KT):
                    nc.tensor.matmul(
                        ps[:],
                        x_sb[:, k, tt * 128 : (tt + 1) * 128],
                        wv_sb[:, k, :],
                        start=(k == 0),
                        stop=(k == KT - 1),
                    )
                nc.vector.tensor_scalar_mul(v_sb[:, tt, :], ps[:], 1.0)

        # ---- Phase B: attention + output projection ----
        with ExitStack() as pb:
            stp = pb.enter_context(tc.tile_pool(name="stp", bufs=2, space="PSUM"))
            avp = pb.enter_context(tc.tile_pool(name="avp", bufs=1, space="PSUM"))
            onp = pb.enter_context(tc.tile_pool(name="onp", bufs=1, space="PSUM"))
            opp = pb.enter_context(tc.tile_pool(name="opp", bufs=2, space="PSUM"))
            ptp = pb.enter_context(tc.tile_pool(name="ptp", bufs=3))
            smp = pb.enter_context(tc.tile_pool(name="smp", bufs=2))
            nrm = pb.enter_context(tc.tile_pool(name="nrm", bufs=2))
            avs = pb.enter_context(tc.tile_pool(name="avs", bufs=2))
            att = pb.enter_context(tc.tile_pool(name="att", bufs=2))
            outp = pb.enter_context(tc.tile_pool(name="outp", bufs=3))

            def attn_head(g, h, att_g):
                nblocks = 4 * g + 4
                npairs = nblocks // 2
                av = avp.tile([128, GW], F32, tag="av")
                on = onp.tile([1, GW], F32, tag="on")
                gcol0 = g * GW

                def emit_pair(jp, g=g, h=h):
                    st2 = stp.tile([128, 2 * GW], F32, tag="st")
                    pt2 = ptp.tile([128, 2 * GW], BF, tag="pt")
                    c0s = []
                    for jl in (0, 1):
                        j = 2 * jp + jl
                        di = j - 4 * g
                        c0 = di * 128 if di >= 0 else 0
                        c0s.append(c0)
                        nc.tensor.matmul(
                            st2[:, jl * GW + c0 : (jl + 1) * GW],
                            kTt[:, h, j * 128 : (j + 1) * 128],
                            qT[:, h, gcol0 + c0 : gcol0 + GW],
                            start=True,
                            stop=True,
                        )
                        if di >= 0:
                            nc.vector.tensor_tensor(
                                st2[:, jl * GW + c0 : jl * GW + c0 + 128],
                                st2[:, jl * GW + c0 : jl * GW + c0 + 128],
                                mask_sb[:],
                                ALU.add,
                            )
                    if c0s[1] == 0:
                        nc.scalar.activation(pt2[:], st2[:], AF.Exp)
                    else:
                        for jl in (0, 1):
                            c0 = c0s[jl]
                            nc.scalar.activation(
                                pt2[:, jl * GW + c0 : (jl + 1) * GW],
                                st2[:, jl * GW + c0 : (jl + 1) * GW],
                                AF.Exp,
                            )
                    return pt2, c0s

                def consume(jp, pt2, c0s, h=h):
                    for jl in (0, 1):
                        j = 2 * jp + jl
                        c0 = c0s[jl]
                        sl = slice(jl * GW + c0, (jl + 1) * GW)
                        nc.tensor.matmul(
                            av[:, c0:GW],
                            v_sb[:, j, h * 128 : (h + 1) * 128],
                            pt2[:, sl],
                            start=(j == 0),
                            stop=(j == nblocks - 1),
                        )
                        nc.tensor.matmul(
                            on[0:1, c0:GW],
                            ones_sb[:],
                            pt2[:, sl],
                            start=(j == 0),
                            stop=(j == nblocks - 1),
                        )

                pend = []
                for jp in range(npairs):
                    pend.append((jp, *emit_pair(jp)))
                    if len(pend) > 1:
                        consume(*pend.pop(0))
                for item in pend:
                    consume(*item)

                av_sb = avs.tile([128, GW], F32, tag="avsb")
                nc.vector.tensor_scalar_mul(av_sb[:], av[:], 1.0)
                on_sb = smp.tile([1, GW], F32, tag="onsb")
                nc.scalar.copy(on_sb[0:1, :], on[0:1, :])
                scr = smp.tile([1, GW], F32, tag="scr")
                ri1 = smp.tile([1, GW], F32, tag="ri1")
                nc.vector.reciprocal_approx_accurate(
                    ri1[0:1, :], on_sb[0:1, :], scr[0:1, :]
                )
                ri = nrm.tile([128, GW], F32, tag="ri")
                nc.gpsimd.partition_broadcast(ri[:], ri1[0:1, :])
                nc.vector.tensor_tensor(
                    att_g[:, h, :], av_sb[:], ri[:], ALU.mult
                )

            def oproj(g, att_g):
                for tloc in range(4):
                    tt = g * 4 + tloc
                    for cch in range(C // GW):
                        op = opp.tile([128, GW], F32, tag="op")
                        for kc in range(HPC):
                            nc.tensor.matmul(
                                op[:],
                                att_g[:, kc, tloc * 128 : (tloc + 1) * 128],
                                wo_sb[:, kc, cch * GW : (cch + 1) * GW],
                                start=(kc == 0),
                                stop=(kc == HPC - 1),
                            )
                        osb = outp.tile([128, GW], F32, tag="osb")
                        nc.vector.tensor_scalar_mul(osb[:], op[:], 1.0)
                        nc.sync.dma_start(
                            out[
                                tt * 128 : (tt + 1) * 128,
                                cch * GW : (cch + 1) * GW,
                            ],
                            osb[:],
                        )

            prev = None
            for g in range(NG):
                att_g = att.tile([128, HPC, GW], BF, tag="att", name=f"att_{g}")
                for h in range(HPC):
                    attn_head(g, h, att_g)
                if prev is not None:
                    oproj(*prev)
                prev = (g, att_g)
            oproj(*prev)

    nc.compile()
    return nc


def _rope_tables(T_, theta=THETA):
    inv = 1.0 / (theta ** (np.arange(0, D, 2, dtype=np.float64) / D))
    t = np.arange(T_, dtype=np.float64)
    fr = np.outer(t, inv)
    emb = np.concatenate([fr, fr], axis=1)
    return (
        np.cos(emb).T.astype(np.float32).copy(),
        np.sin(emb).T.astype(np.float32).copy(),
    )


def _maskT():
    tk = np.arange(128)[:, None]
    c = np.arange(128)[None, :]
    return np.where(c >= tk, 0.0, NEG).astype(np.float32)


def prep_inputs(x, Wq, bq, Wk, bk, Wv, bv, Wo, bo):
    scale = 1.0 / math.sqrt(D)
    cosT, sinT = _rope_tables(T)
    sinMv = sinT.copy()
    sinMv[: D // 2] = -sinMv[: D // 2]
    maskT = _maskT()
    ones = np.ones((128, 1), dtype=BF_NP)
    cosT = cosT.astype(BF_NP)
    sinMv = sinMv.astype(BF_NP)
    xT = [np.ascontiguousarray(x[b].T).astype(BF_NP) for b in range(B)]
    in_maps = []
    for c in range(N_CORES):
        b, g = c // GROUPS, c % GROUPS
        rows = slice(g * HPC * D, (g + 1) * HPC * D)
        in_maps.append(
            {
                "xT": xT[b],
                "wqT": np.ascontiguousarray((Wq[rows] * scale).T).astype(BF_NP),
                "wkT": np.ascontiguousarray(Wk[rows].T).astype(BF_NP),
                "wvT": np.ascontiguousarray(Wv[rows].T).astype(BF_NP),
                "woT": np.ascontiguousarray(Wo[:, rows].T).astype(BF_NP),
                "bq": np.ascontiguousarray(bq[rows] * scale).astype(np.float32),
                "bk": np.ascontiguousarray(bk[rows]).astype(np.float32),
                "cosT": cosT,
                "sinM": sinMv,
                "maskT": maskT,
                "ones": ones,
            }
        )
    bo_eff = (bo + bv @ Wo.T).astype(np.float32)
    return in_maps, bo_eff


_NC_CACHE = {}


def get_nc():
    if "nc" not in _NC_CACHE:
        _NC_CACHE["nc"] = build_core_nc()
    return _NC_CACHE["nc"]


def kernel(x, Wq, bq, Wk, bk, Wv, bv, Wo, bo):
    x = np.asarray(x, dtype=np.float32)
    args = [np.asarray(a, dtype=np.float32) for a in (Wq, bq, Wk, bk, Wv, bv, Wo, bo)]
    in_maps, bo_eff = prep_inputs(x, *args)
    nc = get_nc()

    from concourse.bass_utils import run_bass_kernel_spmd

    res = run_bass_kernel_spmd(nc, in_maps, core_ids=list(range(N_CORES))).results

    out = np.empty((B, T, C), dtype=np.float32)
    for b in range(B):
        acc_ = res[b * GROUPS]["out"].astype(np.float32).copy()
        for g in range(1, GROUPS):
            acc_ += res[b * GROUPS + g]["out"]
        out[b] = acc_ + bo_eff
    return out


# revision 6
# speedup vs baseline: 1.1762x; 1.0989x over previous
"""Trainium2 Bass kernel for causal multi-head attention with RoPE.

Problem: B=2, T=2048, C=2048, H=16, D=128.
Sharding over 8 NeuronCores: batch (2) x head-group (4 heads each); the host
sums the 4 per-head-group partials per batch and adds bo' = bo + bv @ Wo.T
(the v-bias commutes through softmax since rows sum to 1).

v3.1 design notes:
- All matmuls bf16 (fp8 is numerically fatal here: the softmax is peaked
  enough that fp8 x alone gives 4% output error).
- x staged as 64 [128,512] tiles (per k-slab x T-chunk) so the first matmul
  only waits on 2MB of DMA, not the full 8MB.
- Q/K projections: h-outer, k-outer, chunk-inner with 4 live chunk-psums;
  the stationary weight tile is reused across 4 matmuls so LDWEIGHTS has 4x
  the slack to hide (measured 46ns/mm exposure otherwise).
- V projection split: tt0..7 in phase A, tt8..11 / tt12..15 + O-proj blocks
  used as PE spacers between attention groups so the scalar engine's exp
  (the attention pacing engine) gets catch-up windows and the PE never
  idles long enough to re-throttle the HAM clock gate.
- RoPE off the PE: rotate-half via SBUF->SBUF DMA partition swap (sign
  folded into the sin table), 2 DVE mults + GPSIMD add.
- Max-free softmax, scores pre-transposed ST=[tk,tq]; exp batched [128,1024]
  per j-pair; ones-vector matmul row-sums in PSUM.
- PSUM budget exactly 8 banks in each phase: A: qk ring 8x[128,512];
  B: ST pairs 2x2 + av 1 + ones 1 + op/V-spacer ring 2.
"""

import math
import sys

import numpy as np

for _p in ("/opt/trn_rl_repo", "/root/.axon_site/_ro/trn_rl_repo"):
    if _p not in sys.path:
        sys.path.append(_p)

import ml_dtypes

import concourse.bacc as bacc
import concourse.bass as bass
import concourse.mybir as mybir
import concourse.tile as tile
from contextlib import ExitStack

F32 = mybir.dt.float32
BF = mybir.dt.bfloat16
AF = mybir.ActivationFunctionType
ALU = mybir.AluOpType

B, T, C = 2, 2048, 2048
H, D = 16, 128
THETA = 10000.0
NEG = -1e9

N_CORES = 8
GROUPS = 4          # head groups (other shard axis is batch)
HPC = H // GROUPS   # heads per core
KT = C // 128       # contraction k-slabs
GW = 512            # T-group width
NG = T // GW        # attention groups
NT = T // 128       # 128-wide t-tiles
NCH = T // GW       # x column chunks
BF_NP = ml_dtypes.bfloat16


def build_core_nc(debug=False):
    nc = bacc.Bacc(None, target_bir_lowering=False, debug=debug)

    xT = nc.dram_tensor("xT", [C, T], BF, kind="ExternalInput")
    wqT = nc.dram_tensor("wqT", [C, HPC * 128], BF, kind="ExternalInput")
    wkT = nc.dram_tensor("wkT", [C, HPC * 128], BF, kind="ExternalInput")
    wvT = nc.dram_tensor("wvT", [C, HPC * 128], BF, kind="ExternalInput")
    woT = nc.dram_tensor("woT", [HPC * 128, C], BF, kind="ExternalInput")
    bq = nc.dram_tensor("bq", [HPC * 128], F32, kind="ExternalInput")
    bk = nc.dram_tensor("bk", [HPC * 128], F32, kind="ExternalInput")
    cosT = nc.dram_tensor("cosT", [128, T], BF, kind="ExternalInput")
    sinM = nc.dram_tensor("sinM", [128, T], BF, kind="ExternalInput")
    maskT = nc.dram_tensor("maskT", [128, 128], F32, kind="ExternalInput")
    ones = nc.dram_tensor("ones", [128, 1], BF, kind="ExternalInput")
    out = nc.dram_tensor("out", [T, C], F32, kind="ExternalOutput")

    with tile.TileContext(nc) as tc, ExitStack() as top:
        const = top.enter_context(tc.tile_pool(name="const", bufs=1))
        big = top.enter_context(tc.tile_pool(name="big", bufs=1))

        bq_sb = const.tile([128, HPC], F32, name="bq_sb")
        nc.sync.dma_start(bq_sb[:], bq.rearrange("(h d) -> d h", d=128))
        bk_sb = const.tile([128, HPC], F32, name="bk_sb")
        nc.sync.dma_start(bk_sb[:], bk.rearrange("(h d) -> d h", d=128))
        mask_sb = const.tile([128, 128], F32, name="mask_sb")
        nc.sync.dma_start(mask_sb[:], maskT[:, :])
        ones_sb = const.tile([128, 1], BF, name="ones_sb")
        nc.sync.dma_start(ones_sb[:], ones[:, :])

        # Phase-A-scoped pool opens early: weight tiles die with phase A.
        pa = ExitStack()
        paw = pa.enter_context(tc.tile_pool(name="paw", bufs=1))

        # Stationary weight tiles for K then Q: 8 x [128, KT, 128].
        wqk = []
        for i, (wdram, nm) in enumerate(((wkT, "k"), (wqT, "q"))):
            for h in range(HPC):
                wt = paw.tile([128, KT, 128], BF, name=f"w_{nm}{h}")
                nc.sync.dma_start(
                    wt[:],
                    wdram[:, h * 128 : (h + 1) * 128].rearrange(
                        "(ko ki) n -> ki ko n", ki=128
                    ),
                )
                wqk.append(wt)
                if i == 0 and h == 0:
                    # x chunk 0 right after the first weight tile
                    x_t = [[None] * NCH for _ in range(KT)]
                    for k in range(KT):
                        x_t[k][0] = big.tile([128, GW], BF, name=f"x_{k}_0")
                        nc.sync.dma_start(
                            x_t[k][0][:],
                            xT[k * 128 : (k + 1) * 128, 0:GW],
                        )
                    cos_sb = const.tile([128, T], BF, name="cos_sb")
                    nc.sync.dma_start(cos_sb[:], cosT[:, :])
                    sin_sb = const.tile([128, T], BF, name="sin_sb")
                    nc.sync.dma_start(sin_sb[:], sinM[:, :])
        for ch in range(1, NCH):
            for k in range(KT):
                x_t[k][ch] = big.tile([128, GW], BF, name=f"x_{k}_{ch}")
                nc.sync.dma_start(
                    x_t[k][ch][:],
                    xT[k * 128 : (k + 1) * 128, ch * GW : (ch + 1) * GW],
                )
        wv_sb = big.tile([128, KT, HPC * 128], BF, name="wv_sb")
        nc.sync.dma_start(
            wv_sb[:], wvT.rearrange("(ko ki) n -> ki ko n", ki=128)
        )
        wo_sb = big.tile([128, HPC, C], BF, name="wo_sb")
        nc.sync.dma_start(
            wo_sb[:], woT.rearrange("(ho hi) c -> hi ho c", hi=128)
        )
        qT = big.tile([128, HPC, T], BF, name="qT")
        kTt = big.tile([128, HPC, T], BF, name="kTt")
        v_sb = big.tile([128, NT, HPC * 128], BF, name="v_sb")

        def vproj_tt(tt, pool, tag):
            ps = pool.tile([128, HPC * 128], F32, tag=tag, name=f"vps_{tt}")
            for k in range(KT):
                nc.tensor.matmul(
                    ps[:],
                    x_t[k][tt // 4][:, (tt % 4) * 128 : (tt % 4 + 1) * 128],
                    wv_sb[:, k, :],
                    start=(k == 0),
                    stop=(k == KT - 1),
                )
            nc.vector.tensor_scalar_mul(v_sb[:, tt, :], ps[:], 1.0)

        # ---- Phase A: Q/K projections + V tt0..7 ----
        with pa:
            qkp = pa.enter_context(tc.tile_pool(name="qkp", bufs=8, space="PSUM"))
            raw = pa.enter_context(tc.tile_pool(name="raw", bufs=3))

            for i, (bias_sb, dstT) in enumerate(((bk_sb, kTt), (bq_sb, qT))):
                for h in range(HPC):
                    wt = wqk[i * HPC + h]
                    psums = [
                        qkp.tile([128, GW], F32, tag="qk", name=f"qk_{i}_{h}_{ch}")
                        for ch in range(NCH)
                    ]
                    for k in range(KT):
                        for ch in range(NCH):
                            nc.tensor.matmul(
                                psums[ch][:],
                                wt[:, k, :],
                                x_t[k][ch][:],
                                start=(k == 0),
                                stop=(k == KT - 1),
                            )
                    for ch in range(NCH):
                        cols = slice(ch * GW, (ch + 1) * GW)
                        rawt = raw.tile([128, GW], BF, tag="raw")
                        nc.scalar.activation(
                            rawt[:], psums[ch][:], AF.Identity,
                            bias=bias_sb[:, h : h + 1],
                        )
                        swap = raw.tile([128, GW], BF, tag="swap")
                        nc.sync.dma_start(swap[0:64, :], rawt[64:128, :])
                        nc.sync.dma_start(swap[64:128, :], rawt[0:64, :])
                        t1 = raw.tile([128, GW], BF, tag="t1")
                        nc.vector.tensor_tensor(
                            t1[:], rawt[:], cos_sb[:, cols], ALU.mult
                        )
                        t2 = raw.tile([128, GW], BF, tag="t2")
                        nc.vector.tensor_tensor(
                            t2[:], swap[:], sin_sb[:, cols], ALU.mult
                        )
                        nc.gpsimd.tensor_tensor(
                            dstT[:, h, cols], t1[:], t2[:], ALU.add
                        )

            for tt in range(8):
                vproj_tt(tt, qkp, "qk")

        # ---- Phase B: attention + V tt8..15 + output projection ----
        with ExitStack() as pb:
            stp = pb.enter_context(tc.tile_pool(name="stp", bufs=2, space="PSUM"))
            avp = pb.enter_context(tc.tile_pool(name="avp", bufs=1, space="PSUM"))
            onp = pb.enter_context(tc.tile_pool(name="onp", bufs=1, space="PSUM"))
            opp = pb.enter_context(tc.tile_pool(name="opp", bufs=2, space="PSUM"))
            ptp = pb.enter_context(tc.tile_pool(name="ptp", bufs=3))
            smp = pb.enter_context(tc.tile_pool(name="smp", bufs=1))
            nrm = pb.enter_context(tc.tile_pool(name="nrm", bufs=2))
            avs = pb.enter_context(tc.tile_pool(name="avs", bufs=2))
            att = pb.enter_context(tc.tile_pool(name="att", bufs=3))
            outp = pb.enter_context(tc.tile_pool(name="outp", bufs=2))

            def attn_head(g, h, att_g):
                nblocks = 4 * g + 4
                npairs = nblocks // 2
                av = avp.tile([128, GW], F32, tag="av")
                on = onp.tile([1, GW], F32, tag="on")
                gcol0 = g * GW

                def emit_pair(jp, g=g, h=h):
                    st2 = stp.tile([128, 2 * GW], F32, tag="st")
                    pt2 = ptp.tile([128, 2 * GW], BF, tag="pt")
                    c0s = []
                    for jl in (0, 1):
                        j = 2 * jp + jl
                        di = j - 4 * g
                        c0 = di * 128 if di >= 0 else 0
                        c0s.append(c0)
                        nc.tensor.matmul(
                            st2[:, jl * GW + c0 : (jl + 1) * GW],
                            kTt[:, h, j * 128 : (j + 1) * 128],
                            qT[:, h, gcol0 + c0 : gcol0 + GW],
                            start=True,
                            stop=True,
                        )
                        if di >= 0:
                            nc.vector.tensor_tensor(
                                st2[:, jl * GW + c0 : jl * GW + c0 + 128],
                                st2[:, jl * GW + c0 : jl * GW + c0 + 128],
                                mask_sb[:],
                                ALU.add,
                            )
                    if c0s[1] == 0:
                        nc.scalar.activation(pt2[:], st2[:], AF.Exp)
                    else:
                        for jl in (0, 1):
                            c0 = c0s[jl]
                            nc.scalar.activation(
                                pt2[:, jl * GW + c0 : (jl + 1) * GW],
                                st2[:, jl * GW + c0 : (jl + 1) * GW],
                                AF.Exp,
                            )
                    return pt2, c0s

                def consume(jp, pt2, c0s, h=h):
                    for jl in (0, 1):
                        j = 2 * jp + jl
                        c0 = c0s[jl]
                        sl = slice(jl * GW + c0, (jl + 1) * GW)
                        nc.tensor.matmul(
                            av[:, c0:GW],
                            v_sb[:, j, h * 128 : (h + 1) * 128],
                            pt2[:, sl],
                            start=(j == 0),
                            stop=(j == nblocks - 1),
                        )
                        nc.tensor.matmul(
                            on[0:1, c0:GW],
                            ones_sb[:],
                            pt2[:, sl],
                            start=(j == 0),
                            stop=(j == nblocks - 1),
                        )

                pend = []
                for jp in range(npairs):
                    pend.append((jp, *emit_pair(jp)))
                    if len(pend) > 1:
                        consume(*pend.pop(0))
                for item in pend:
                    consume(*item)

                av_sb = avs.tile([128, GW], F32, tag="avsb")
                nc.vector.tensor_scalar_mul(av_sb[:], av[:], 1.0)
                on_sb = smp.tile([1, GW], F32, tag="onsb")
                nc.scalar.copy(on_sb[0:1, :], on[0:1, :])
                scr = smp.tile([1, GW], F32, tag="scr")
                ri1 = smp.tile([1, GW], F32, tag="ri1")
                nc.vector.reciprocal_approx_accurate(
                    ri1[0:1, :], on_sb[0:1, :], scr[0:1, :]
                )
                ri = nrm.tile([128, GW], F32, tag="ri")
                nc.gpsimd.partition_broadcast(ri[:], ri1[0:1, :])
                nc.vector.tensor_tensor(
                    att_g[:, h, :], av_sb[:], ri[:], ALU.mult
                )

            def oproj(g, att_g):
                for tloc in range(4):
                    tt = g * 4 + tloc
                    for cch in range(C // GW):
                        op = opp.tile([128, GW], F32, tag="op")
                        for kc in range(HPC):
                            nc.tensor.matmul(
                                op[:],
                                att_g[:, kc, tloc * 128 : (tloc + 1) * 128],
                                wo_sb[:, kc, cch * GW : (cch + 1) * GW],
                                start=(kc == 0),
                                stop=(kc == HPC - 1),
                            )
                        osb = outp.tile([128, GW], F32, tag="osb")
                        nc.scalar.copy(osb[:], op[:])
                        nc.sync.dma_start(
                            out[
                                tt * 128 : (tt + 1) * 128,
                                cch * GW : (cch + 1) * GW,
                            ],
                            osb[:],
                        )

            att_tiles = {}

            def attn_group(g):
                att_g = att.tile(
                    [128, HPC, GW], BF, tag="att", name=f"att_{g}"
                )
                att_tiles[g] = att_g
                for h in range(HPC):
                    attn_head(g, h, att_g)

            # schedule with PE spacers between attention groups
            attn_group(0)
            for tt in range(8, 12):
                vproj_tt(tt, opp, "op")
            attn_group(1)
            for tt in range(12, 16):
                vproj_tt(tt, opp, "op")
            attn_group(2)
            oproj(0, att_tiles[0])
            oproj(1, att_tiles[1])
            attn_group(3)
            oproj(2, att_tiles[2])
            oproj(3, att_tiles[3])

    nc.compile()
    return nc


def _rope_tables(T_, theta=THETA):
    inv = 1.0 / (theta ** (np.arange(0, D, 2, dtype=np.float64) / D))
    t = np.arange(T_, dtype=np.float64)
    fr = np.outer(t, inv)
    emb = np.concatenate([fr, fr], axis=1)
    return (
        np.cos(emb).T.astype(np.float32).copy(),
        np.sin(emb).T.astype(np.float32).copy(),
    )


def _maskT():
    tk = np.arange(128)[:, None]
    c = np.arange(128)[None, :]
    return np.where(c >= tk, 0.0, NEG).astype(np.float32)


def prep_inputs(x, Wq, bq, Wk, bk, Wv, bv, Wo, bo):
    scale = 1.0 / math.sqrt(D)
    cosT, sinT = _rope_tables(T)
    sinMv = sinT.copy()
    sinMv[: D // 2] = -sinMv[: D // 2]
    maskT = _maskT()
    ones = np.ones((128, 1), dtype=BF_NP)
    cosT = cosT.astype(BF_NP)
    sinMv = sinMv.astype(BF_NP)
    xT = [np.ascontiguousarray(x[b].T).astype(BF_NP) for b in range(B)]
    in_maps = []
    for c in range(N_CORES):
        b, g = c // GROUPS, c % GROUPS
        rows = slice(g * HPC * D, (g + 1) * HPC * D)
        in_maps.append(
            {
                "xT": xT[b],
                "wqT": np.ascontiguousarray((Wq[rows] * scale).T).astype(BF_NP),
                "wkT": np.ascontiguousarray(Wk[rows].T).astype(BF_NP),
                "wvT": np.ascontiguousarray(Wv[rows].T).astype(BF_NP),
                "woT": np.ascontiguousarray(Wo[:, rows].T).astype(BF_NP),
                "bq": np.ascontiguousarray(bq[rows] * scale).astype(np.float32),
                "bk": np.ascontiguousarray(bk[rows]).astype(np.float32),
                "cosT": cosT,
                "sinM": sinMv,
                "maskT": maskT,
                "ones": ones,
            }
        )
    bo_eff = (bo + bv @ Wo.T).astype(np.float32)
    return in_maps, bo_eff


_NC_CACHE = {}


def get_nc():
    if "nc" not in _NC_CACHE:
        _NC_CACHE["nc"] = build_core_nc()
    return _NC_CACHE["nc"]


def kernel(x, Wq, bq, Wk, bk, Wv, bv, Wo, bo):
    x = np.asarray(x, dtype=np.float32)
    args = [np.asarray(a, dtype=np.float32) for a in (Wq, bq, Wk, bk, Wv, bv, Wo, bo)]
    in_maps, bo_eff = prep_inputs(x, *args)
    nc = get_nc()

    from concourse.bass_utils import run_bass_kernel_spmd

    res = run_bass_kernel_spmd(nc, in_maps, core_ids=list(range(N_CORES))).results

    out = np.empty((B, T, C), dtype=np.float32)
    for b in range(B):
        acc_ = res[b * GROUPS]["out"].astype(np.float32).copy()
        for g in range(1, GROUPS):
            acc_ += res[b * GROUPS + g]["out"]
        out[b] = acc_ + bo_eff
    return out


# revision 12
# speedup vs baseline: 1.2515x; 1.0640x over previous
"""Trainium2 Bass kernel for causal multi-head attention with RoPE.

Problem: B=2, T=2048, C=2048, H=16, D=128.
Sharding over 8 NeuronCores: batch (2) x head-group (4 heads each); the host
sums the 4 per-head-group partials per batch and adds bo' = bo + bv @ Wo.T
(the v-bias commutes through softmax since rows sum to 1).

v3.1 design notes:
- All matmuls bf16 (fp8 is numerically fatal here: the softmax is peaked
  enough that fp8 x alone gives 4% output error).
- x staged as 64 [128,512] tiles (per k-slab x T-chunk) so the first matmul
  only waits on 2MB of DMA, not the full 8MB.
- Q/K projections: h-outer, k-outer, chunk-inner with 4 live chunk-psums;
  the stationary weight tile is reused across 4 matmuls so LDWEIGHTS has 4x
  the slack to hide (measured 46ns/mm exposure otherwise).
- V projection split: tt0..7 in phase A, tt8..11 / tt12..15 + O-proj blocks
  used as PE spacers between attention groups so the scalar engine's exp
  (the attention pacing engine) gets catch-up windows and the PE never
  idles long enough to re-throttle the HAM clock gate.
- RoPE off the PE: rotate-half via SBUF->SBUF DMA partition swap (sign
  folded into the sin table), 2 DVE mults + GPSIMD add.
- Max-free softmax, scores pre-transposed ST=[tk,tq]; exp batched [128,1024]
  per j-pair; ones-vector matmul row-sums in PSUM.
- PSUM budget exactly 8 banks in each phase: A: qk ring 8x[128,512];
  B: ST pairs 2x2 + av 1 + ones 1 + op/V-spacer ring 2.
"""

import math
import sys

import numpy as np

for _p in ("/opt/trn_rl_repo", "/root/.axon_site/_ro/trn_rl_repo"):
    if _p not in sys.path:
        sys.path.append(_p)

import ml_dtypes

import concourse.bacc as bacc
import concourse.bass as bass
import concourse.mybir as mybir
import concourse.tile as tile
from contextlib import ExitStack

F32 = mybir.dt.float32
BF = mybir.dt.bfloat16
AF = mybir.ActivationFunctionType
ALU = mybir.AluOpType

B, T, C = 2, 2048, 2048
H, D = 16, 128
THETA = 10000.0
NEG = -1e9

N_CORES = 8
GROUPS = 4          # head groups (other shard axis is batch)
HPC = H // GROUPS   # heads per core
KT = C // 128       # contraction k-slabs
GW = 512            # T-group width
NG = T // GW        # attention groups
NT = T // 128       # 128-wide t-tiles
NCH = T // GW       # x column chunks
BF_NP = ml_dtypes.bfloat16


def build_core_nc(debug=False):
    nc = bacc.Bacc(None, target_bir_lowering=False, debug=debug)

    xT = nc.dram_tensor("xT", [C, T], BF, kind="ExternalInput")
    wqT = nc.dram_tensor("wqT", [C, HPC * 128], BF, kind="ExternalInput")
    wkT = nc.dram_tensor("wkT", [C, HPC * 128], BF, kind="ExternalInput")
    wvT = nc.dram_tensor("wvT", [C, HPC * 128], BF, kind="ExternalInput")
    woT = nc.dram_tensor("woT", [HPC * 128, C], BF, kind="ExternalInput")
    bq = nc.dram_tensor("bq", [HPC * 128], F32, kind="ExternalInput")
    bk = nc.dram_tensor("bk", [HPC * 128], F32, kind="ExternalInput")
    cosT = nc.dram_tensor("cosT", [128, T], BF, kind="ExternalInput")
    sinM = nc.dram_tensor("sinM", [128, T], BF, kind="ExternalInput")
    maskT = nc.dram_tensor("maskT", [128, 128], F32, kind="ExternalInput")
    ones = nc.dram_tensor("ones", [128, 1], BF, kind="ExternalInput")
    out = nc.dram_tensor("out", [T, C], F32, kind="ExternalOutput")

    with tile.TileContext(nc) as tc, ExitStack() as top:
        const = top.enter_context(tc.tile_pool(name="const", bufs=1))
        big = top.enter_context(tc.tile_pool(name="big", bufs=1))

        bq_sb = const.tile([128, HPC], F32, name="bq_sb")
        nc.sync.dma_start(bq_sb[:], bq.rearrange("(h d) -> d h", d=128))
        bk_sb = const.tile([128, HPC], F32, name="bk_sb")
        nc.sync.dma_start(bk_sb[:], bk.rearrange("(h d) -> d h", d=128))
        mask_sb = const.tile([128, 128], F32, name="mask_sb")
        nc.sync.dma_start(mask_sb[:], maskT[:, :])
        ones_sb = const.tile([128, 1], BF, name="ones_sb")
        nc.sync.dma_start(ones_sb[:], ones[:, :])

        # Phase-A-scoped pool opens early: weight tiles die with phase A.
        pa = ExitStack()
        paw = pa.enter_context(tc.tile_pool(name="paw", bufs=1))

        # Stationary weight tiles for K then Q: 8 x [128, KT, 128].
        wqk = []
        for i, (wdram, nm) in enumerate(((wkT, "k"), (wqT, "q"))):
            for h in range(HPC):
                wt = paw.tile([128, KT, 128], BF, name=f"w_{nm}{h}")
                nc.sync.dma_start(
                    wt[:],
                    wdram[:, h * 128 : (h + 1) * 128].rearrange(
                        "(ko ki) n -> ki ko n", ki=128
                    ),
                )
                wqk.append(wt)
                if i == 0 and h == 0:
                    # x chunk 0 right after the first weight tile
                    x_t = [[None] * NCH for _ in range(KT)]
                    for k in range(KT):
                        x_t[k][0] = big.tile([128, GW], BF, name=f"x_{k}_0")
                        nc.sync.dma_start(
                            x_t[k][0][:],
                            xT[k * 128 : (k + 1) * 128, 0:GW],
                        )
                    cos_sb = const.tile([128, T], BF, name="cos_sb")
                    nc.sync.dma_start(cos_sb[:], cosT[:, :])
                    sin_sb = const.tile([128, T], BF, name="sin_sb")
                    nc.sync.dma_start(sin_sb[:], sinM[:, :])
        wv_sb = big.tile([128, KT, HPC * 128], BF, name="wv_sb")
        nc.sync.dma_start(
            wv_sb[:], wvT.rearrange("(ko ki) n -> ki ko n", ki=128)
        )
        for ch in range(1, NCH):
            for k in range(KT):
                x_t[k][ch] = big.tile([128, GW], BF, name=f"x_{k}_{ch}")
                nc.sync.dma_start(
                    x_t[k][ch][:],
                    xT[k * 128 : (k + 1) * 128, ch * GW : (ch + 1) * GW],
                )
        wo_sb = big.tile([128, HPC, C], BF, name="wo_sb")
        nc.sync.dma_start(
            wo_sb[:], woT.rearrange("(ho hi) c -> hi ho c", hi=128)
        )
        qT = big.tile([128, HPC, T], BF, name="qT")
        kTt = big.tile([128, HPC, T], BF, name="kTt")
        v_sb = big.tile([128, NT, HPC * 128], BF, name="v_sb")

        def vproj_tt(tt, pool, tag):
            ps = pool.tile([128, HPC * 128], F32, tag=tag, name=f"vps_{tt}")
            for k in range(KT):
                nc.tensor.matmul(
                    ps[:],
                    x_t[k][tt // 4][:, (tt % 4) * 128 : (tt % 4 + 1) * 128],
                    wv_sb[:, k, :],
                    start=(k == 0),
                    stop=(k == KT - 1),
                )
            nc.scalar.copy(v_sb[:, tt, :], ps[:])

        # ---- Phase A: Q/K projections + V tt0..7 ----
        with pa:
            qkp = pa.enter_context(tc.tile_pool(name="qkp", bufs=8, space="PSUM"))
            raw = pa.enter_context(tc.tile_pool(name="raw", bufs=3))

            def rope_evac(ps, ch, bias_sb, dstT, h):
                cols = slice(ch * GW, (ch + 1) * GW)
                rawt = raw.tile([128, GW], BF, tag="raw")
                nc.scalar.activation(
                    rawt[:], ps[:], AF.Identity, bias=bias_sb[:, h : h + 1]
                )
                swap = raw.tile([128, GW], BF, tag="swap")
                nc.sync.dma_start(swap[0:64, :], rawt[64:128, :])
                nc.sync.dma_start(swap[64:128, :], rawt[0:64, :])
                t1 = raw.tile([128, GW], BF, tag="t1")
                nc.vector.tensor_tensor(t1[:], rawt[:], cos_sb[:, cols], ALU.mult)
                t2 = raw.tile([128, GW], BF, tag="t2")
                nc.vector.tensor_tensor(t2[:], swap[:], sin_sb[:, cols], ALU.mult)
                nc.gpsimd.tensor_tensor(dstT[:, h, cols], t1[:], t2[:], ALU.add)

            # chunk 0 alone first: only 2MB of x needed, PE starts while the
            # rest of x streams in
            for i, (bias_sb, dstT) in enumerate(((bk_sb, kTt), (bq_sb, qT))):
                for h in range(HPC):
                    wt = wqk[i * HPC + h]
                    ps = qkp.tile([128, GW], F32, tag="qk", name=f"qk0_{i}_{h}")
                    for k in range(KT):
                        nc.tensor.matmul(
                            ps[:], wt[:, k, :], x_t[k][0][:],
                            start=(k == 0), stop=(k == KT - 1),
                        )
                    rope_evac(ps, 0, bias_sb, dstT, h)

            for tt in range(4):
                vproj_tt(tt, qkp, "qk")

            # chunks 1..3 as triples: stationary reused across 3 matmuls
            for i, (bias_sb, dstT) in enumerate(((bk_sb, kTt), (bq_sb, qT))):
                for h in range(HPC):
                    wt = wqk[i * HPC + h]
                    psums = [
                        qkp.tile([128, GW], F32, tag="qk", name=f"qk_{i}_{h}_{ch}")
                        for ch in range(1, NCH)
                    ]
                    for k in range(KT):
                        for ci, ch in enumerate(range(1, NCH)):
                            nc.tensor.matmul(
                                psums[ci][:],
                                wt[:, k, :],
                                x_t[k][ch][:],
                                start=(k == 0),
                                stop=(k == KT - 1),
                            )
                    for ci, ch in enumerate(range(1, NCH)):
                        rope_evac(psums[ci], ch, bias_sb, dstT, h)

            for tt in range(4, 8):
                vproj_tt(tt, qkp, "qk")

        # ---- Phase B: attention + V tt8..15 + output projection ----
        with ExitStack() as pb:
            stp = pb.enter_context(tc.tile_pool(name="stp", bufs=2, space="PSUM"))
            avp = pb.enter_context(tc.tile_pool(name="avp", bufs=1, space="PSUM"))
            onp = pb.enter_context(tc.tile_pool(name="onp", bufs=1, space="PSUM"))
            opp = pb.enter_context(tc.tile_pool(name="opp", bufs=2, space="PSUM"))
            ptp = pb.enter_context(tc.tile_pool(name="ptp", bufs=3))
            smp = pb.enter_context(tc.tile_pool(name="smp", bufs=1))
            nrm = pb.enter_context(tc.tile_pool(name="nrm", bufs=2))
            avs = pb.enter_context(tc.tile_pool(name="avs", bufs=2))
            att = pb.enter_context(tc.tile_pool(name="att", bufs=3))
            outp = pb.enter_context(tc.tile_pool(name="outp", bufs=2))

            def attn_head(g, h, att_g):
                nblocks = 4 * g + 4
                npairs = nblocks // 2
                av = avp.tile([128, GW], F32, tag="av")
                on = onp.tile([1, GW], F32, tag="on")
                gcol0 = g * GW

                def emit_pair(jp, g=g, h=h):
                    st2 = stp.tile([128, 2 * GW], F32, tag="st")
                    pt2 = ptp.tile([128, 2 * GW], BF, tag="pt")
                    c0s = []
                    for jl in (0, 1):
                        j = 2 * jp + jl
                        di = j - 4 * g
                        c0 = di * 128 if di >= 0 else 0
                        c0s.append(c0)
                        nc.tensor.matmul(
                            st2[:, jl * GW + c0 : (jl + 1) * GW],
                            kTt[:, h, j * 128 : (j + 1) * 128],
                            qT[:, h, gcol0 + c0 : gcol0 + GW],
                            start=True,
                            stop=True,
                        )
                        if di >= 0:
                            nc.vector.tensor_tensor(
                                st2[:, jl * GW + c0 : jl * GW + c0 + 128],
                                st2[:, jl * GW + c0 : jl * GW + c0 + 128],
                                mask_sb[:],
                                ALU.add,
                            )
                    if c0s[1] == 0:
                        nc.scalar.activation(pt2[:], st2[:], AF.Exp)
                    else:
                        for jl in (0, 1):
                            c0 = c0s[jl]
                            nc.scalar.activation(
                                pt2[:, jl * GW + c0 : (jl + 1) * GW],
                                st2[:, jl * GW + c0 : (jl + 1) * GW],
                                AF.Exp,
                            )
                    return pt2, c0s

                def consume(jp, pt2, c0s, h=h):
                    for jl in (0, 1):
                        j = 2 * jp + jl
                        c0 = c0s[jl]
                        sl = slice(jl * GW + c0, (jl + 1) * GW)
                        nc.tensor.matmul(
                            av[:, c0:GW],
                            v_sb[:, j, h * 128 : (h + 1) * 128],
                            pt2[:, sl],
                            start=(j == 0),
                            stop=(j == nblocks - 1),
                        )
                        nc.tensor.matmul(
                            on[0:1, c0:GW],
                            ones_sb[:],
                            pt2[:, sl],
                            start=(j == 0),
                            stop=(j == nblocks - 1),
                        )

                pend = []
                for jp in range(npairs):
                    pend.append((jp, *emit_pair(jp)))
                    if len(pend) > 1:
                        consume(*pend.pop(0))
                for item in pend:
                    consume(*item)

                av_sb = avs.tile([128, GW], F32, tag="avsb")
                nc.vector.tensor_scalar_mul(av_sb[:], av[:], 1.0)
                on_sb = smp.tile([1, GW], F32, tag="onsb")
                nc.scalar.copy(on_sb[0:1, :], on[0:1, :])
                ri1 = smp.tile([1, GW], F32, tag="ri1")
                nc.vector.reciprocal_approx_fast(ri1[0:1, :], on_sb[0:1, :])
                ri = nrm.tile([128, GW], F32, tag="ri")
                nc.gpsimd.partition_broadcast(ri[:], ri1[0:1, :])
                nc.vector.tensor_tensor(
                    att_g[:, h, :], av_sb[:], ri[:], ALU.mult
                )

            def oproj(g, att_g, lo=0, hi=16):
                for idx in range(lo, hi):
                    tloc, cch = idx // 4, idx % 4
                    tt = g * 4 + tloc
                    op = opp.tile([128, GW], F32, tag="op")
                    for kc in range(HPC):
                        nc.tensor.matmul(
                            op[:],
                            att_g[:, kc, tloc * 128 : (tloc + 1) * 128],
                            wo_sb[:, kc, cch * GW : (cch + 1) * GW],
                            start=(kc == 0),
                            stop=(kc == HPC - 1),
                        )
                    osb = outp.tile([128, GW], F32, tag="osb")
                    nc.scalar.copy(osb[:], op[:])
                    nc.sync.dma_start(
                        out[
                            tt * 128 : (tt + 1) * 128,
                            cch * GW : (cch + 1) * GW,
                        ],
                        osb[:],
                    )

            att_tiles = {}

            def attn_group(g):
                att_g = att.tile(
                    [128, HPC, GW], BF, tag="att", name=f"att_{g}"
                )
                att_tiles[g] = att_g
                for h in range(HPC):
                    attn_head(g, h, att_g)

            # schedule with PE spacers between attention groups; G3's heads
            # interleave with O-proj half-blocks so exp always has PE cover
            attn_group(0)
            for tt in range(8, 12):
                vproj_tt(tt, opp, "op")
            attn_group(1)
            for tt in range(12, 16):
                vproj_tt(tt, opp, "op")
            attn_group(2)
            oproj(0, att_tiles[0])
            att_g3 = att.tile([128, HPC, GW], BF, tag="att", name="att_3")
            att_tiles[3] = att_g3
            attn_head(3, 0, att_g3)
            oproj(1, att_tiles[1], 0, 8)
            attn_head(3, 1, att_g3)
            oproj(1, att_tiles[1], 8, 16)
            attn_head(3, 2, att_g3)
            oproj(2, att_tiles[2], 0, 8)
            attn_head(3, 3, att_g3)
            oproj(2, att_tiles[2], 8, 16)
            oproj(3, att_tiles[3])

    nc.compile()
    return nc


def _rope_tables(T_, theta=THETA):
    inv = 1.0 / (theta ** (np.arange(0, D, 2, dtype=np.float64) / D))
    t = np.arange(T_, dtype=np.float64)
    fr = np.outer(t, inv)
    emb = np.concatenate([fr, fr], axis=1)
    return (
        np.cos(emb).T.astype(np.float32).copy(),
        np.sin(emb).T.astype(np.float32).copy(),
    )


def _maskT():
    tk = np.arange(128)[:, None]
    c = np.arange(128)[None, :]
    return np.where(c >= tk, 0.0, NEG).astype(np.float32)


def prep_inputs(x, Wq, bq, Wk, bk, Wv, bv, Wo, bo):
    scale = 1.0 / math.sqrt(D)
    cosT, sinT = _rope_tables(T)
    sinMv = sinT.copy()
    sinMv[: D // 2] = -sinMv[: D // 2]
    maskT = _maskT()
    ones = np.ones((128, 1), dtype=BF_NP)
    cosT = cosT.astype(BF_NP)
    sinMv = sinMv.astype(BF_NP)
    xT = [np.ascontiguousarray(x[b].T).astype(BF_NP) for b in range(B)]
    in_maps = []
    for c in range(N_CORES):
        b, g = c // GROUPS, c % GROUPS
        rows = slice(g * HPC * D, (g + 1) * HPC * D)
        in_maps.append(
            {
                "xT": xT[b],
                "wqT": np.ascontiguousarray((Wq[rows] * scale).T).astype(BF_NP),
                "wkT": np.ascontiguousarray(Wk[rows].T).astype(BF_NP),
                "wvT": np.ascontiguousarray(Wv[rows].T).astype(BF_NP),
                "woT": np.ascontiguousarray(Wo[:, rows].T).astype(BF_NP),
                "bq": np.ascontiguousarray(bq[rows] * scale).astype(np.float32),
                "bk": np.ascontiguousarray(bk[rows]).astype(np.float32),
                "cosT": cosT,
                "sinM": sinMv,
                "maskT": maskT,
                "ones": ones,
            }
        )
    bo_eff = (bo + bv @ Wo.T).astype(np.float32)
    return in_maps, bo_eff


_NC_CACHE = {}


def get_nc():
    if "nc" not in _NC_CACHE:
        _NC_CACHE["nc"] = build_core_nc()
    return _NC_CACHE["nc"]


def kernel(x, Wq, bq, Wk, bk, Wv, bv, Wo, bo):
    x = np.asarray(x, dtype=np.float32)
    args = [np.asarray(a, dtype=np.float32) for a in (Wq, bq, Wk, bk, Wv, bv, Wo, bo)]
    in_maps, bo_eff = prep_inputs(x, *args)
    nc = get_nc()

    from concourse.bass_utils import run_bass_kernel_spmd

    res = run_bass_kernel_spmd(nc, in_maps, core_ids=list(range(N_CORES))).results

    out = np.empty((B, T, C), dtype=np.float32)
    for b in range(B):
        acc_ = res[b * GROUPS]["out"].astype(np.float32).copy()
        for g in range(1, GROUPS):
            acc_ += res[b * GROUPS + g]["out"]
        out[b] = acc_ + bo_eff
    return out
